# revision 3
# baseline (speedup 1.0000x reference)
"""Trainium2 Bass kernel for nn_DiffusionBlock (anisotropic diffusion step).

Math (per batch-channel image; s = tau*hx^2, hx = grad kernel tap):
  X[i,j] = u[i,j+1]-u[i,j] (0 at j=W-1),  Y[i,j] = u[i+1,j]-u[i,j] (0 at i=H-1)
  XP/YP  = edge-pad(X/Y) on the (H+2, W+2) grid
  F = a*XP + b*YP,  G = b*XP + c*YP              (padded grid)
  out[i,j] = u[i,j] + s*(F[i+1,j+1]-F[i+1,j] + G[i+1,j+1]-G[i,j+1])

Pure batch data-parallel across 8 cores (1 batch each). The per-call cost
through the axon tunnel is dominated by (a) per-operand dispatch overhead
(~2 ms/operand) and (b) shipped input bytes (~18 GB/s), so inputs are packed
into TWO tensors in reduced precision (tolerance 2e-2, measured ~4e-3):
  uw  bf16 [n_ch*H + 128, W]: u rows (ch-major) + 128 rows of PE weights
  abc fp8 E3M4 [n_ch, H+2, 3*(W+2)]: per-row concat a|b|c
      (b in [0,0.1] sits in E3M4's subnormal range: abs err <= 2^-7,
       contribution ~0.1% of output norm)
  out bf16 (upcast to f32 on host)

Per-core layout: row-tiles of R=126 output rows. SBUF partition q holds:
  U[q]  = u row r0-1+q (top edge-clamped)     [rt+1, W]  bf16
  U2[q] = u row r0+q   (bottom edge-clamped)  [rt+1, W]  bf16
  ABC[q] = a|b|c row r0+q                     [rt+1, 3*(W+2)] fp8
All gradients/products on DVE (partition-aligned):
  XT = free-dim diff of U;  YT = U2 - U (partition-offset loads)
  F[q,s] = A*XPc + B*YPc   (padded-grid row r0+q, cols 0..W)
  G2[q,j] = G[r0+q, j+1] = B*XT + C*YT
PE assembles the divergence in PSUM with 3 constant bf16 weight matrices
(partition shift / sign / scale s folded in):
  OUTP[p] = s*(F[p+1]@j+1 - F[p+1]@j + G2[p+1] - G2[p])
DVE adds U2 (PSUM read) -> OS bf16, DMA stores. Top/bottom clamps are
folded into the DMA row loads (replicated rows); the first tile fixes
YT[0] = YT[1] with a 1-partition SBUF copy.
"""

import numpy as np
import ml_dtypes

# Problem geometry (hardcoded per harness contract).
N_CORES = 8
N_CH = 2
H = 1024
W = 1024
R = 126       # output rows per tile
CHUNK = 512   # matmul free-dim chunk (= one PSUM bank of fp32)

BF16 = ml_dtypes.bfloat16
F8E3 = ml_dtypes.float8_e3m4

_W_NAMES = ("wsp", "wsn", "wg")
W_ROWS = N_CH * H          # uw row where the weight block starts


def _host_weights(s: float):
    """Constant PE weight matrices, packed [128, 3*128] bf16.

    matmul(out, lhsT, rhs): out[p, n] = sum_k lhsT[k, p] * rhs[k, n]
    """
    k = np.arange(128)[:, None]
    p = np.arange(128)[None, :]
    sf = np.float32(s)
    wsp = sf * (k == p + 1)                  # out[p] += s * x[p+1]
    wsn = -sf * (k == p + 1)                 # out[p] -= s * x[p+1]
    wg = sf * (k == p + 1) - sf * (k == p)   # out[p] += s * (x[p+1]-x[p])
    mats = {"wsp": wsp, "wsn": wsn, "wg": wg}
    return np.ascontiguousarray(
        np.concatenate([mats[n].astype(np.float32) for n in _W_NAMES], axis=1)
    ).astype(BF16)


def _build_nc(n_ch: int, h: int, w: int, r: int, chunk: int, reps: int = 1, mode: str = "full"):
    import concourse.bacc as bacc
    import concourse.mybir as mybir
    import concourse.tile as tile

    f32 = mybir.dt.float32
    bf16 = mybir.dt.bfloat16
    f8e3 = mybir.dt.float8e3

    nwt = len(_W_NAMES) * 128
    w_rows = n_ch * h
    nc = bacc.Bacc(enable_partition_id=False)
    uw_d = nc.dram_tensor("uw", [w_rows + 128, w], bf16, kind="ExternalInput")
    abc_d = nc.dram_tensor(
        "abc", [n_ch, h + 2, 3 * (w + 2)], f8e3, kind="ExternalInput"
    )
    out_d = nc.dram_tensor("out", [n_ch, h, w], bf16, kind="ExternalOutput")

    tiles = [(r0, min(r, h - r0)) for r0 in range(0, h, r)]

    with tile.TileContext(nc) as tc:
        with (
            tc.tile_pool(name="wpool", bufs=1) as wpool,
            tc.tile_pool(name="io", bufs=3) as io,
            tc.tile_pool(name="tmp", bufs=2) as tmp,
            tc.tile_pool(name="psum", bufs=2, space="PSUM") as psum,
        ):
            # one DMA for the weights, then a tiny high-priority matmul so PE
            # observes the weights DMA once up front (matmul sync-wait slots
            # are scarce; no per-tile matmul then carries that wait)
            w_all = wpool.tile([128, nwt], bf16, tag="w_all")
            nc.sync.dma_start(w_all[:], uw_d[w_rows : w_rows + 128, 0:nwt])
            wt = {
                n: w_all[:, i * 128 : (i + 1) * 128]
                for i, n in enumerate(_W_NAMES)
            }
            warm = psum.tile([1, 4], f32, tag="OUTP")
            with tc.high_priority():
                nc.tensor.matmul(warm[0:1, 0:1], w_all[0:1, 0:1], w_all[0:1, 0:1])

            for _rep in range(reps):
              for ch in range(n_ch):
                for r0, rt in tiles:
                    first = r0 == 0
                    last = r0 + rt == h
                    ka = rt + 1      # ABC/XT/YT/F/G partitions
                    # ---- loads ----
                    # U[q] = u row r0-1+q (top-clamped)
                    U = io.tile([128, w], bf16, tag="U")
                    lo = r0 - 1
                    clo = max(lo, 0)
                    nc.sync.dma_start(
                        U[clo - lo : ka, :], uw_d[ch * h + clo : ch * h + lo + ka, :]
                    )
                    if first:
                        nc.sync.dma_start(U[0:1, :], uw_d[ch * h : ch * h + 1, :])
                    # U2[q] = u row r0+q (bottom-clamped)
                    U2 = io.tile([128, w], bf16, tag="U2")
                    hi = min(r0 + ka, h)
                    nc.sync.dma_start(
                        U2[0 : hi - r0, :], uw_d[ch * h + r0 : ch * h + hi, :]
                    )
                    if last:
                        nc.sync.dma_start(
                            U2[ka - 1 : ka, :],
                            uw_d[ch * h + h - 1 : ch * h + h, :],
                        )
                    # a|b|c rows r0..r0+rt: one contiguous-row DMA
                    ABC = io.tile([128, 3 * (w + 2)], f8e3, tag="ABC")
                    nc.sync.dma_start(ABC[0:ka, :], abc_d[ch, r0 : r0 + ka, :])
                    A = ABC[:, 0 : w + 2]
                    Bt = ABC[:, w + 2 : 2 * (w + 2)]
                    C = ABC[:, 2 * (w + 2) : 3 * (w + 2)]

                    do_dve = mode in ("full", "nomm")
                    do_pe = mode in ("full", "nodve")
                    # ---- gradients (DVE) ----
                    # XT[q] = X row r0-1+q: free-dim forward diff, col W-1 = 0
                    XT = tmp.tile([128, w], bf16, tag="XT")
                    YT = tmp.tile([128, w], bf16, tag="YT")
                    if do_dve:
                        nc.vector.tensor_sub(
                            XT[0:ka, 0 : w - 1], U[0:ka, 1:w], U[0:ka, 0 : w - 1]
                        )
                        nc.vector.memset(XT[0:ka, w - 1 : w], 0.0)
                        # YT[q] = Y row r0-1+q = U2 - U (partition-offset loads)
                        nc.vector.tensor_sub(YT[0:ka, :], U2[0:ka, :], U[0:ka, :])
                        if first:
                            # YT[0] = Y[-1] -> clamp = Y[0] (= YT[1])
                            nc.sync.dma_start(YT[0:1, :], YT[1:2, :])

                    # ---- products (DVE) ----
                    # F[q,s] = a[r0+q,s]*XP[r0+q,s] + b[r0+q,s]*YP[r0+q,s]
                    #   XP/YP col s -> XT/YT local col s-1 (clamped at s=0)
                    F = tmp.tile([128, w + 1], bf16, tag="F")
                    T = tmp.tile([128, w + 1], bf16, tag="T")
                    G2 = tmp.tile([128, w], bf16, tag="G2")
                    T2 = tmp.tile([128, w], bf16, tag="T2")
                    if do_dve:
                        nc.vector.tensor_mul(
                            F[0:ka, 1 : w + 1], A[0:ka, 1 : w + 1], XT[0:ka, 0:w]
                        )
                        nc.vector.tensor_mul(F[0:ka, 0:1], A[0:ka, 0:1], XT[0:ka, 0:1])
                        nc.vector.tensor_mul(
                            T[0:ka, 1 : w + 1], Bt[0:ka, 1 : w + 1], YT[0:ka, 0:w]
                        )
                        nc.vector.tensor_mul(T[0:ka, 0:1], Bt[0:ka, 0:1], YT[0:ka, 0:1])
                        nc.vector.tensor_add(F[0:ka, :], F[0:ka, :], T[0:ka, :])
                        # G2[q,j] = G[r0+q, j+1]
                        nc.vector.tensor_mul(
                            G2[0:ka, 0:w], Bt[0:ka, 1 : w + 1], XT[0:ka, 0:w]
                        )
                        nc.vector.tensor_mul(
                            T2[0:ka, 0:w], C[0:ka, 1 : w + 1], YT[0:ka, 0:w]
                        )
                        nc.vector.tensor_add(G2[0:ka, :], G2[0:ka, :], T2[0:ka, :])

                    # ---- PSUM assembly (PE) ----
                    # OUTP[p] = s*(F[p+1]@j+1 - F[p+1]@j + G2[p+1] - G2[p])
                    OUTP = psum.tile([128, w], f32, tag="OUTP")
                    for n0 in (range(0, w, chunk) if do_pe else ()):
                        cw = min(chunk, w - n0)
                        o = OUTP[0:rt, n0 : n0 + cw]
                        mm = [
                            (wt["wsp"][0:ka, 0:rt], F[0:ka, n0 + 1 : n0 + 1 + cw]),
                            (wt["wsn"][0:ka, 0:rt], F[0:ka, n0 : n0 + cw]),
                            (wt["wg"][0:ka, 0:rt], G2[0:ka, n0 : n0 + cw]),
                        ]
                        for i, (lhsT, rhs) in enumerate(mm):
                            nc.tensor.matmul(
                                o,
                                lhsT,
                                rhs,
                                start=(i == 0),
                                stop=(i == len(mm) - 1),
                            )

                    # ---- out = U2 + OUTP (DVE, PSUM read), store bf16 ----
                    OS = tmp.tile([128, w], bf16, tag="OS")
                    if do_pe and do_dve:
                        nc.vector.tensor_add(OS[0:rt, :], OUTP[0:rt, :], U2[0:rt, :])
                    else:
                        nc.vector.memset(OS[0:1, 0:4], 0.0)
                        if do_dve:
                            nc.vector.memset(OUTP[0:1, 0:4], 0.0)
                        if do_pe:
                            for _t in (F, G2):
                                nc.vector.memset(_t[0:1, 0:4], 0.0)
                    nc.sync.dma_start(out_d[ch, r0 : r0 + rt, :], OS[0:rt, :])

    nc.compile()
    return nc


def prepare_inputs(u, a, b, c, tau, grad_x, grad_y):
    """Host-side casts + packing. Returns (uw, abc) full-batch arrays."""
    hx = float(np.asarray(grad_x)[0, 0, 1, 2])
    s = float(np.asarray(tau)) * hx * hx
    wts = _host_weights(s)
    u8 = np.ascontiguousarray(np.asarray(u, dtype=np.float32)).astype(BF16)
    a8 = np.ascontiguousarray(np.asarray(a, dtype=np.float32)).astype(F8E3)
    b8 = np.ascontiguousarray(np.asarray(b, dtype=np.float32)).astype(F8E3)
    c8 = np.ascontiguousarray(np.asarray(c, dtype=np.float32)).astype(F8E3)
    n_cores = u8.shape[0]
    wblock = np.zeros((128, W), BF16)
    wblock[:, : wts.shape[1]] = wts
    uw = np.concatenate(
        [u8.reshape(n_cores, N_CH * H, W), np.broadcast_to(wblock, (n_cores, 128, W))],
        axis=1,
    )
    abc = np.concatenate([a8, b8, c8], axis=3)
    return np.ascontiguousarray(uw), np.ascontiguousarray(abc)


def kernel(u, a, b, c, tau, grad_x, grad_y):
    from concourse.bass_utils import run_bass_kernel_spmd

    uw, abc = prepare_inputs(u, a, b, c, tau, grad_x, grad_y)
    nc = _build_nc(N_CH, H, W, R, CHUNK)
    in_maps = [{"uw": uw[k], "abc": abc[k]} for k in range(N_CORES)]
    res = run_bass_kernel_spmd(nc, in_maps, list(range(N_CORES)))
    return np.stack(
        [res.results[k]["out"].astype(np.float32) for k in range(N_CORES)], axis=0
    )


# revision 4
# speedup vs baseline: 1.6542x; 1.6542x over previous
"""Trainium2 Bass kernel for nn_DiffusionBlock (anisotropic diffusion step).

Math (per batch-channel image; s = tau*hx^2, hx = grad kernel tap):
  X[i,j] = u[i,j+1]-u[i,j] (0 at j=W-1),  Y[i,j] = u[i+1,j]-u[i,j] (0 at i=H-1)
  XP/YP  = edge-pad(X/Y) on the (H+2, W+2) grid
  F = a*XP + b*YP,  G = b*XP + c*YP              (padded grid)
  out[i,j] = u[i,j] + s*(F[i+1,j+1]-F[i+1,j] + G[i+1,j+1]-G[i,j+1])

Pure batch data-parallel across 8 cores (1 batch each). The per-call cost
through the axon tunnel is dominated by per-operand dispatch overhead and
shipped input bytes, so ALL inputs are packed into ONE fp8 E3M4 tensor of
1024-byte rows (tolerance 2e-2, measured ~5e-3):
  rows [0, 2048):      u (ch-major image rows)
  rows [2048, +2052*3): a, b, c planes, padded-grid cols 1..W only
                        (b in [0,0.1] sits in E3M4's fine subnormal range)
  rows [8204, 8332):    3 PE weight matrices, bf16 bytes bitcast-loaded
The device computes the diffusion DELTA (s * divergence) in bf16; the host
adds the exact f32 u and recomputes output column 0 exactly (that column
needs a/b at padded col 0, which the packed planes drop).

Per-core layout: row-tiles of R=126 output rows. SBUF partition q holds:
  U[q]  = u row r0-1+q (top edge-clamped)     [rt+1, W]  fp8
  U2[q] = u row r0+q   (bottom edge-clamped)  [rt+1, W]  fp8
  A/B/C[q] = plane row r0+q (padded cols 1..W)[rt+1, W]  fp8
All gradients/products on DVE (partition-aligned):
  XT = free-dim diff of U;  YT = U2 - U (partition-offset loads)
  F[q,s] = A*XT + B*YT at padded row r0+q, cols 1..W (col 0 zeroed)
  G2[q,j] = G[r0+q, j+1] = B*XT + C*YT
PE assembles the divergence in PSUM with 3 constant bf16 weight matrices
(partition shift / sign / scale s folded in):
  DELTA[p] = s*(F[p+1]@j+1 - F[p+1]@j + G2[p+1] - G2[p])
ACT copies PSUM -> bf16, DMA stores. Top/bottom clamps are folded into the
DMA row loads (replicated rows); the first tile fixes YT[0] = YT[1] with a
1-partition SBUF copy.
"""

import numpy as np
import ml_dtypes

# Problem geometry (hardcoded per harness contract).
N_CORES = 8
N_CH = 2
H = 1024
W = 1024
R = 126       # output rows per tile
CHUNK = 512   # matmul free-dim chunk (= one PSUM bank of fp32)

BF16 = ml_dtypes.bfloat16
F8E3 = ml_dtypes.float8_e3m4

_W_NAMES = ("wsp", "wsn", "wg")
# packed-tensor row bases
ROW_A = N_CH * H
ROW_B = ROW_A + N_CH * (H + 2)
ROW_C = ROW_B + N_CH * (H + 2)
ROW_W = ROW_C + N_CH * (H + 2)
N_ROWS = ROW_W + 128


def _host_weights(s: float):
    """Constant PE weight matrices, packed [128, 3*128] bf16.

    matmul(out, lhsT, rhs): out[p, n] = sum_k lhsT[k, p] * rhs[k, n]
    """
    k = np.arange(128)[:, None]
    p = np.arange(128)[None, :]
    sf = np.float32(s)
    wsp = sf * (k == p + 1)                  # out[p] += s * x[p+1]
    wsn = -sf * (k == p + 1)                 # out[p] -= s * x[p+1]
    wg = sf * (k == p + 1) - sf * (k == p)   # out[p] += s * (x[p+1]-x[p])
    mats = {"wsp": wsp, "wsn": wsn, "wg": wg}
    return np.ascontiguousarray(
        np.concatenate([mats[n].astype(np.float32) for n in _W_NAMES], axis=1)
    ).astype(BF16)


def _build_nc(n_ch: int, h: int, w: int, r: int, chunk: int, reps: int = 1, mode: str = "full"):
    import concourse.bacc as bacc
    import concourse.mybir as mybir
    import concourse.tile as tile

    f32 = mybir.dt.float32
    bf16 = mybir.dt.bfloat16
    f8e3 = mybir.dt.float8e3

    nwt = len(_W_NAMES) * 128
    nc = bacc.Bacc(enable_partition_id=False)
    all_d = nc.dram_tensor("all", [N_ROWS, w], f8e3, kind="ExternalInput")
    out_d = nc.dram_tensor("out", [n_ch, h, w], bf16, kind="ExternalOutput")

    tiles = [(r0, min(r, h - r0)) for r0 in range(0, h, r)]

    with tile.TileContext(nc) as tc:
        with (
            tc.tile_pool(name="wpool", bufs=1) as wpool,
            tc.tile_pool(name="io", bufs=3) as io,
            tc.tile_pool(name="tmp", bufs=2) as tmp,
            tc.tile_pool(name="psum", bufs=2, space="PSUM") as psum,
        ):
            # one DMA for the weights (fp8 rows bitcast back to bf16), then a
            # tiny high-priority matmul so PE observes the weights DMA once up
            # front (matmul sync-wait slots are scarce)
            w_all = wpool.tile([128, nwt], bf16, tag="w_all")
            nc.sync.dma_start(
                w_all[:], all_d[ROW_W : ROW_W + 128, 0 : 2 * nwt].bitcast(bf16)
            )
            wt = {
                n: w_all[:, i * 128 : (i + 1) * 128]
                for i, n in enumerate(_W_NAMES)
            }
            warm = psum.tile([1, 4], f32, tag="DELTA")
            with tc.high_priority():
                nc.tensor.matmul(warm[0:1, 0:1], w_all[0:1, 0:1], w_all[0:1, 0:1])

            for _rep in range(reps):
              for ch in range(n_ch):
                for r0, rt in tiles:
                    first = r0 == 0
                    last = r0 + rt == h
                    ka = rt + 1      # working partitions
                    # ---- loads ----
                    # U[q] = u row r0-1+q (top-clamped)
                    U = io.tile([128, w], f8e3, tag="U")
                    lo = r0 - 1
                    clo = max(lo, 0)
                    nc.sync.dma_start(
                        U[clo - lo : ka, :], all_d[ch * h + clo : ch * h + lo + ka, :]
                    )
                    if first:
                        nc.sync.dma_start(U[0:1, :], all_d[ch * h : ch * h + 1, :])
                    # U2[q] = u row r0+q (bottom-clamped)
                    U2 = io.tile([128, w], f8e3, tag="U2")
                    hi = min(r0 + ka, h)
                    nc.sync.dma_start(
                        U2[0 : hi - r0, :], all_d[ch * h + r0 : ch * h + hi, :]
                    )
                    if last:
                        nc.sync.dma_start(
                            U2[ka - 1 : ka, :],
                            all_d[ch * h + h - 1 : ch * h + h, :],
                        )
                    # plane rows r0..r0+rt (padded-grid cols 1..W)
                    hp = h + 2
                    A = io.tile([128, w], f8e3, tag="A")
                    Bt = io.tile([128, w], f8e3, tag="B")
                    C = io.tile([128, w], f8e3, tag="C")
                    nc.sync.dma_start(
                        A[0:ka, :], all_d[ROW_A + ch * hp + r0 : ROW_A + ch * hp + r0 + ka, :]
                    )
                    nc.sync.dma_start(
                        Bt[0:ka, :], all_d[ROW_B + ch * hp + r0 : ROW_B + ch * hp + r0 + ka, :]
                    )
                    nc.sync.dma_start(
                        C[0:ka, :], all_d[ROW_C + ch * hp + r0 : ROW_C + ch * hp + r0 + ka, :]
                    )

                    do_dve = mode in ("full", "nomm")
                    do_pe = mode in ("full", "nodve")
                    # ---- gradients (DVE) ----
                    # XT[q] = X row r0-1+q: free-dim forward diff, col W-1 = 0
                    XT = tmp.tile([128, w], bf16, tag="XT")
                    YT = tmp.tile([128, w], bf16, tag="YT")
                    if do_dve:
                        nc.vector.tensor_sub(
                            XT[0:ka, 0 : w - 1], U[0:ka, 1:w], U[0:ka, 0 : w - 1]
                        )
                        nc.vector.memset(XT[0:ka, w - 1 : w], 0.0)
                        # YT[q] = Y row r0-1+q = U2 - U (partition-offset loads)
                        nc.vector.tensor_sub(YT[0:ka, :], U2[0:ka, :], U[0:ka, :])
                        if first:
                            # YT[0] = Y[-1] -> clamp = Y[0] (= YT[1])
                            nc.sync.dma_start(YT[0:1, :], YT[1:2, :])

                    # ---- products (DVE) ----
                    # F[q,s] = a[r0+q,s]*XT[q,s-1] + b[r0+q,s]*YT[q,s-1],
                    #   s in 1..W (planes hold cols 1..W at local col s-1);
                    #   F col 0 zeroed (host recomputes out col 0 exactly)
                    F = tmp.tile([128, w + 1], bf16, tag="F")
                    T = tmp.tile([128, w], bf16, tag="T")
                    G2 = tmp.tile([128, w], bf16, tag="G2")
                    T2 = tmp.tile([128, w], bf16, tag="T2")
                    if do_dve:
                        nc.vector.tensor_mul(F[0:ka, 1 : w + 1], A[0:ka, :], XT[0:ka, :])
                        nc.vector.memset(F[0:ka, 0:1], 0.0)
                        nc.vector.tensor_mul(T[0:ka, :], Bt[0:ka, :], YT[0:ka, :])
                        nc.vector.tensor_add(
                            F[0:ka, 1 : w + 1], F[0:ka, 1 : w + 1], T[0:ka, :]
                        )
                        # G2[q,j] = G[r0+q, j+1]
                        nc.vector.tensor_mul(G2[0:ka, :], Bt[0:ka, :], XT[0:ka, :])
                        nc.vector.tensor_mul(T2[0:ka, :], C[0:ka, :], YT[0:ka, :])
                        nc.vector.tensor_add(G2[0:ka, :], G2[0:ka, :], T2[0:ka, :])

                    # ---- PSUM assembly (PE) ----
                    # DELTA[p] = s*(F[p+1]@j+1 - F[p+1]@j + G2[p+1] - G2[p])
                    DELTA = psum.tile([128, w], f32, tag="DELTA")
                    for n0 in (range(0, w, chunk) if do_pe else ()):
                        cw = min(chunk, w - n0)
                        o = DELTA[0:rt, n0 : n0 + cw]
                        mm = [
                            (wt["wsp"][0:ka, 0:rt], F[0:ka, n0 + 1 : n0 + 1 + cw]),
                            (wt["wsn"][0:ka, 0:rt], F[0:ka, n0 : n0 + cw]),
                            (wt["wg"][0:ka, 0:rt], G2[0:ka, n0 : n0 + cw]),
                        ]
                        for i, (lhsT, rhs) in enumerate(mm):
                            nc.tensor.matmul(
                                o,
                                lhsT,
                                rhs,
                                start=(i == 0),
                                stop=(i == len(mm) - 1),
                            )

                    # ---- PSUM -> SBUF bf16 (ACT), store ----
                    OS = tmp.tile([128, w], bf16, tag="OS")
                    if do_pe:
                        nc.scalar.copy(OS[0:rt, :], DELTA[0:rt, :])
                    else:
                        nc.vector.memset(OS[0:1, 0:4], 0.0)
                        if do_dve:
                            for _t in (F, G2):
                                nc.vector.memset(_t[0:1, 0:4], 0.0)
                    nc.sync.dma_start(out_d[ch, r0 : r0 + rt, :], OS[0:rt, :])

    nc.compile()
    return nc


def _scale(tau, grad_x):
    hx = float(np.asarray(grad_x)[0, 0, 1, 2])
    return float(np.asarray(tau)) * hx * hx


def prepare_inputs(u, a, b, c, tau, grad_x, grad_y):
    """Host-side casts + packing into the single fp8 tensor [B, N_ROWS, W]."""
    s = _scale(tau, grad_x)
    wts = _host_weights(s)
    n = np.asarray(u).shape[0]
    u8 = np.ascontiguousarray(np.asarray(u, dtype=np.float32)).astype(F8E3)
    a8 = np.ascontiguousarray(np.asarray(a, np.float32)[:, :, :, 1 : W + 1]).astype(F8E3)
    b8 = np.ascontiguousarray(np.asarray(b, np.float32)[:, :, :, 1 : W + 1]).astype(F8E3)
    c8 = np.ascontiguousarray(np.asarray(c, np.float32)[:, :, :, 1 : W + 1]).astype(F8E3)
    wrow = np.zeros((128, W), F8E3)
    wrow[:, : 2 * wts.shape[1]] = np.ascontiguousarray(wts).view(F8E3)
    packed = np.concatenate(
        [
            u8.reshape(n, N_CH * H, W),
            a8.reshape(n, N_CH * (H + 2), W),
            b8.reshape(n, N_CH * (H + 2), W),
            c8.reshape(n, N_CH * (H + 2), W),
            np.broadcast_to(wrow, (n, 128, W)),
        ],
        axis=1,
    )
    return np.ascontiguousarray(packed)


def postprocess(delta_f32, u, a, b, c, tau, grad_x):
    """out = u + delta, with output column 0 recomputed exactly on host."""
    s = _scale(tau, grad_x)
    u = np.asarray(u, np.float32)
    a = np.asarray(a, np.float32)
    b = np.asarray(b, np.float32)
    c = np.asarray(c, np.float32)
    out = u + delta_f32
    X0 = u[..., 1] - u[..., 0]                        # [B, C, H]
    Y0 = np.zeros_like(X0)
    Y0[..., : H - 1] = u[..., 1:, 0] - u[..., : H - 1, 0]
    rr = np.clip(np.arange(H + 2) - 1, 0, H - 1)
    Xp0, Yp0 = X0[..., rr], Y0[..., rr]               # [B, C, H+2]
    F0 = a[..., 0] * Xp0 + b[..., 0] * Yp0
    F1 = a[..., 1] * Xp0 + b[..., 1] * Yp0
    G1 = b[..., 1] * Xp0 + c[..., 1] * Yp0
    out[..., 0] = u[..., 0] + s * (
        F1[..., 1 : H + 1] - F0[..., 1 : H + 1] + G1[..., 1 : H + 1] - G1[..., 0:H]
    )
    return out


def kernel(u, a, b, c, tau, grad_x, grad_y):
    from concourse.bass_utils import run_bass_kernel_spmd

    packed = prepare_inputs(u, a, b, c, tau, grad_x, grad_y)
    nc = _build_nc(N_CH, H, W, R, CHUNK)
    in_maps = [{"all": packed[k]} for k in range(N_CORES)]
    res = run_bass_kernel_spmd(nc, in_maps, list(range(N_CORES)))
    delta = np.stack(
        [res.results[k]["out"].astype(np.float32) for k in range(N_CORES)], axis=0
    )
    return postprocess(delta, u, a, b, c, tau, grad_x)


# revision 11
# speedup vs baseline: 3.5027x; 2.1174x over previous
"""Trainium2 Bass kernel for nn_DiffusionBlock (anisotropic diffusion step).

Math (per batch-channel image; s = tau*hx^2, hx = grad kernel tap):
  X[i,j] = u[i,j+1]-u[i,j] (0 at j=W-1),  Y[i,j] = u[i+1,j]-u[i,j] (0 at i=H-1)
  XP/YP  = edge-pad(X/Y) on the (H+2, W+2) grid
  F = a*XP + b*YP,  G = b*XP + c*YP              (padded grid)
  out[i,j] = u[i,j] + s*(F[i+1,j+1]-F[i+1,j] + G[i+1,j+1]-G[i,j+1])

Pure batch data-parallel across 8 cores (1 batch each). The per-call cost
through the axon tunnel is dominated by per-operand dispatch overhead and
shipped input bytes, so ALL inputs are packed into ONE fp8 E3M4 tensor of
1024-byte rows (tolerance 2e-2, measured ~5e-3):
  rows [0, 2048):      u (ch-major image rows)
  rows [2048, +2052*3): a, b, c planes, padded-grid cols 1..W only
                        (b in [0,0.1] sits in E3M4's fine subnormal range)
  rows [8204, 8332):    3 PE weight matrices, bf16 bytes bitcast-loaded
The device computes the diffusion DELTA (s * divergence) in bf16; the host
adds the exact f32 u and recomputes output column 0 exactly (that column
needs a/b at padded col 0, which the packed planes drop).

Per-core layout: row-tiles of R=126 output rows. SBUF partition q holds:
  U[q]  = u row r0-1+q (top edge-clamped)     [rt+1, W]  fp8
  U2[q] = u row r0+q   (bottom edge-clamped)  [rt+1, W]  fp8
  A/B/C[q] = plane row r0+q (padded cols 1..W)[rt+1, W]  fp8
All gradients/products on DVE (partition-aligned):
  XT = free-dim diff of U;  YT = U2 - U (partition-offset loads)
  F[q,s] = A*XT + B*YT at padded row r0+q, cols 1..W (col 0 zeroed)
  G2[q,j] = G[r0+q, j+1] = B*XT + C*YT
PE assembles the divergence in PSUM with 3 constant bf16 weight matrices
(partition shift / sign / scale s folded in):
  DELTA[p] = s*(F[p+1]@j+1 - F[p+1]@j + G2[p+1] - G2[p])
ACT copies PSUM -> bf16, DMA stores. Top/bottom clamps are folded into the
DMA row loads (replicated rows); the first tile fixes YT[0] = YT[1] with a
1-partition SBUF copy.
"""

import numpy as np
import ml_dtypes

# Problem geometry (hardcoded per harness contract).
# N_CORES=1 measured fastest: per-call dispatch overhead through the axon
# tunnel scales with core count (~0.3 ms/core), the tunnel byte-pipe is
# shared, and device execution (~0.5 ms for all 8 images) hides entirely
# behind the ~4 ms dispatch+transfer pipeline. 8/4/2/1-core sweep gave
# 6.2/5.5/4.5/4.3 ms marginal per call.
N_CORES = 1
N_CH = 2
H = 1024
W = 1024
R = 126       # output rows per tile
CHUNK = 512   # matmul free-dim chunk (= one PSUM bank of fp32)

BF16 = ml_dtypes.bfloat16
F8E3 = ml_dtypes.float8_e3m4

_W_NAMES = ("wsp", "wsn", "wg")
# packed-tensor row bases
ROW_A = N_CH * H
ROW_B = ROW_A + N_CH * (H + 2)
ROW_C = ROW_B + N_CH * (H + 2)
ROW_W = ROW_C + N_CH * (H + 2)
N_ROWS = ROW_W + 128


def _host_weights(s: float):
    """Constant PE weight matrices, packed [128, 3*128] bf16.

    matmul(out, lhsT, rhs): out[p, n] = sum_k lhsT[k, p] * rhs[k, n]
    """
    k = np.arange(128)[:, None]
    p = np.arange(128)[None, :]
    sf = np.float32(s)
    wsp = sf * (k == p + 1)                  # out[p] += s * x[p+1]
    wsn = -sf * (k == p + 1)                 # out[p] -= s * x[p+1]
    wg = sf * (k == p + 1) - sf * (k == p)   # out[p] += s * (x[p+1]-x[p])
    mats = {"wsp": wsp, "wsn": wsn, "wg": wg}
    return np.ascontiguousarray(
        np.concatenate([mats[n].astype(np.float32) for n in _W_NAMES], axis=1)
    ).astype(BF16)


def _build_nc(n_ch: int, h: int, w: int, r: int, chunk: int, n_img: int = 1,
              reps: int = 1, mode: str = "full"):
    import concourse.bacc as bacc
    import concourse.mybir as mybir
    import concourse.tile as tile

    f32 = mybir.dt.float32
    bf16 = mybir.dt.bfloat16
    f8e3 = mybir.dt.float8e3

    nwt = len(_W_NAMES) * 128
    nc = bacc.Bacc(enable_partition_id=False)
    all_d = nc.dram_tensor(
        "all", [n_img * ROW_W + 128, w], f8e3, kind="ExternalInput"
    )
    out_d = nc.dram_tensor("out", [n_img * n_ch, h, w], bf16, kind="ExternalOutput")

    tiles = [(r0, min(r, h - r0)) for r0 in range(0, h, r)]

    with tile.TileContext(nc) as tc:
        with (
            tc.tile_pool(name="wpool", bufs=1) as wpool,
            tc.tile_pool(name="io", bufs=3) as io,
            tc.tile_pool(name="tmp", bufs=2) as tmp,
            tc.tile_pool(name="psum", bufs=2, space="PSUM") as psum,
        ):
            # one DMA for the weights (fp8 rows bitcast back to bf16), then a
            # tiny high-priority matmul so PE observes the weights DMA once up
            # front (matmul sync-wait slots are scarce)
            w_all = wpool.tile([128, nwt], bf16, tag="w_all")
            wbase = n_img * ROW_W
            nc.sync.dma_start(
                w_all[:], all_d[wbase : wbase + 128, 0 : 2 * nwt].bitcast(bf16)
            )
            wt = {
                n: w_all[:, i * 128 : (i + 1) * 128]
                for i, n in enumerate(_W_NAMES)
            }
            warm = psum.tile([1, 4], f32, tag="DELTA")
            with tc.high_priority():
                nc.tensor.matmul(warm[0:1, 0:1], w_all[0:1, 0:1], w_all[0:1, 0:1])

            for _rep in range(reps):
              for img in range(n_img):
               ib = img * ROW_W
               for ch in range(n_ch):
                for r0, rt in tiles:
                    first = r0 == 0
                    last = r0 + rt == h
                    ka = rt + 1      # working partitions
                    ub = ib + ch * h
                    # ---- loads ----
                    # U[q] = u row r0-1+q (top-clamped)
                    U = io.tile([128, w], f8e3, tag="U")
                    lo = r0 - 1
                    clo = max(lo, 0)
                    nc.sync.dma_start(
                        U[clo - lo : ka, :], all_d[ub + clo : ub + lo + ka, :]
                    )
                    if first:
                        nc.sync.dma_start(U[0:1, :], all_d[ub : ub + 1, :])
                    # U2[q] = u row r0+q (bottom-clamped)
                    U2 = io.tile([128, w], f8e3, tag="U2")
                    hi = min(r0 + ka, h)
                    nc.sync.dma_start(
                        U2[0 : hi - r0, :], all_d[ub + r0 : ub + hi, :]
                    )
                    if last:
                        nc.sync.dma_start(
                            U2[ka - 1 : ka, :],
                            all_d[ub + h - 1 : ub + h, :],
                        )
                    # plane rows r0..r0+rt (padded-grid cols 1..W)
                    hp = h + 2
                    ab = ib + ROW_A + ch * hp + r0
                    bb = ib + ROW_B + ch * hp + r0
                    cb = ib + ROW_C + ch * hp + r0
                    A = io.tile([128, w], f8e3, tag="A")
                    Bt = io.tile([128, w], f8e3, tag="B")
                    C = io.tile([128, w], f8e3, tag="C")
                    nc.sync.dma_start(A[0:ka, :], all_d[ab : ab + ka, :])
                    nc.sync.dma_start(Bt[0:ka, :], all_d[bb : bb + ka, :])
                    nc.sync.dma_start(C[0:ka, :], all_d[cb : cb + ka, :])

                    do_dve = mode in ("full", "nomm")
                    do_pe = mode in ("full", "nodve")
                    # ---- gradients (DVE) ----
                    # XT[q] = X row r0-1+q: free-dim forward diff, col W-1 = 0
                    XT = tmp.tile([128, w], bf16, tag="XT")
                    YT = tmp.tile([128, w], bf16, tag="YT")
                    if do_dve:
                        nc.vector.tensor_sub(
                            XT[0:ka, 0 : w - 1], U[0:ka, 1:w], U[0:ka, 0 : w - 1]
                        )
                        nc.vector.memset(XT[0:ka, w - 1 : w], 0.0)
                        # YT[q] = Y row r0-1+q = U2 - U (partition-offset loads)
                        nc.vector.tensor_sub(YT[0:ka, :], U2[0:ka, :], U[0:ka, :])
                        if first:
                            # YT[0] = Y[-1] -> clamp = Y[0] (= YT[1])
                            nc.sync.dma_start(YT[0:1, :], YT[1:2, :])

                    # ---- products (DVE) ----
                    # F[q,s] = a[r0+q,s]*XT[q,s-1] + b[r0+q,s]*YT[q,s-1],
                    #   s in 1..W (planes hold cols 1..W at local col s-1);
                    #   F col 0 zeroed (host recomputes out col 0 exactly)
                    F = tmp.tile([128, w + 1], bf16, tag="F")
                    T = tmp.tile([128, w], bf16, tag="T")
                    G2 = tmp.tile([128, w], bf16, tag="G2")
                    T2 = tmp.tile([128, w], bf16, tag="T2")
                    if do_dve:
                        nc.vector.tensor_mul(F[0:ka, 1 : w + 1], A[0:ka, :], XT[0:ka, :])
                        nc.vector.memset(F[0:ka, 0:1], 0.0)
                        nc.vector.tensor_mul(T[0:ka, :], Bt[0:ka, :], YT[0:ka, :])
                        nc.vector.tensor_add(
                            F[0:ka, 1 : w + 1], F[0:ka, 1 : w + 1], T[0:ka, :]
                        )
                        # G2[q,j] = G[r0+q, j+1]
                        nc.vector.tensor_mul(G2[0:ka, :], Bt[0:ka, :], XT[0:ka, :])
                        nc.vector.tensor_mul(T2[0:ka, :], C[0:ka, :], YT[0:ka, :])
                        nc.vector.tensor_add(G2[0:ka, :], G2[0:ka, :], T2[0:ka, :])

                    # ---- PSUM assembly (PE) ----
                    # DELTA[p] = s*(F[p+1]@j+1 - F[p+1]@j + G2[p+1] - G2[p])
                    DELTA = psum.tile([128, w], f32, tag="DELTA")
                    for n0 in (range(0, w, chunk) if do_pe else ()):
                        cw = min(chunk, w - n0)
                        o = DELTA[0:rt, n0 : n0 + cw]
                        mm = [
                            (wt["wsp"][0:ka, 0:rt], F[0:ka, n0 + 1 : n0 + 1 + cw]),
                            (wt["wsn"][0:ka, 0:rt], F[0:ka, n0 : n0 + cw]),
                            (wt["wg"][0:ka, 0:rt], G2[0:ka, n0 : n0 + cw]),
                        ]
                        for i, (lhsT, rhs) in enumerate(mm):
                            nc.tensor.matmul(
                                o,
                                lhsT,
                                rhs,
                                start=(i == 0),
                                stop=(i == len(mm) - 1),
                            )

                    # ---- PSUM -> SBUF bf16 (ACT), store ----
                    OS = tmp.tile([128, w], bf16, tag="OS")
                    if do_pe:
                        nc.scalar.copy(OS[0:rt, :], DELTA[0:rt, :])
                    else:
                        nc.vector.memset(OS[0:1, 0:4], 0.0)
                        if do_dve:
                            for _t in (F, G2):
                                nc.vector.memset(_t[0:1, 0:4], 0.0)
                    nc.sync.dma_start(
                        out_d[img * n_ch + ch, r0 : r0 + rt, :], OS[0:rt, :]
                    )

    nc.compile()
    return nc


def _scale(tau, grad_x):
    hx = float(np.asarray(grad_x)[0, 0, 1, 2])
    return float(np.asarray(tau)) * hx * hx


def prepare_inputs(u, a, b, c, tau, grad_x, grad_y, n_cores: int = None):
    """Host casts + packing into one fp8 tensor [n_cores, n_img*ROW_W+128, W]."""
    if n_cores is None:
        n_cores = N_CORES
    s = _scale(tau, grad_x)
    wts = _host_weights(s)
    n = np.asarray(u).shape[0]
    n_img = n // n_cores
    u8 = np.ascontiguousarray(np.asarray(u, dtype=np.float32)).astype(F8E3)
    a8 = np.ascontiguousarray(np.asarray(a, np.float32)[:, :, :, 1 : W + 1]).astype(F8E3)
    b8 = np.ascontiguousarray(np.asarray(b, np.float32)[:, :, :, 1 : W + 1]).astype(F8E3)
    c8 = np.ascontiguousarray(np.asarray(c, np.float32)[:, :, :, 1 : W + 1]).astype(F8E3)
    wrow = np.zeros((128, W), F8E3)
    wrow[:, : 2 * wts.shape[1]] = np.ascontiguousarray(wts).view(F8E3)
    # per-image data block [ROW_W, W]
    data = np.concatenate(
        [
            u8.reshape(n, N_CH * H, W),
            a8.reshape(n, N_CH * (H + 2), W),
            b8.reshape(n, N_CH * (H + 2), W),
            c8.reshape(n, N_CH * (H + 2), W),
        ],
        axis=1,
    )  # [n, ROW_W, W]
    # core k gets images k*n_img .. (k+1)*n_img-1, then the weight block
    packed = np.concatenate(
        [
            data.reshape(n_cores, n_img * ROW_W, W),
            np.broadcast_to(wrow, (n_cores, 128, W)),
        ],
        axis=1,
    )
    return np.ascontiguousarray(packed)


def postprocess(delta_f32, u, a, b, c, tau, grad_x):
    """out = u + delta, with output column 0 recomputed exactly on host."""
    s = _scale(tau, grad_x)
    u = np.asarray(u, np.float32)
    a = np.asarray(a, np.float32)
    b = np.asarray(b, np.float32)
    c = np.asarray(c, np.float32)
    out = u + delta_f32
    X0 = u[..., 1] - u[..., 0]                        # [B, C, H]
    Y0 = np.zeros_like(X0)
    Y0[..., : H - 1] = u[..., 1:, 0] - u[..., : H - 1, 0]
    rr = np.clip(np.arange(H + 2) - 1, 0, H - 1)
    Xp0, Yp0 = X0[..., rr], Y0[..., rr]               # [B, C, H+2]
    F0 = a[..., 0] * Xp0 + b[..., 0] * Yp0
    F1 = a[..., 1] * Xp0 + b[..., 1] * Yp0
    G1 = b[..., 1] * Xp0 + c[..., 1] * Yp0
    out[..., 0] = u[..., 0] + s * (
        F1[..., 1 : H + 1] - F0[..., 1 : H + 1] + G1[..., 1 : H + 1] - G1[..., 0:H]
    )
    return out


def kernel(u, a, b, c, tau, grad_x, grad_y):
    from concourse.bass_utils import run_bass_kernel_spmd

    n = np.asarray(u).shape[0]
    n_img = n // N_CORES
    packed = prepare_inputs(u, a, b, c, tau, grad_x, grad_y, N_CORES)
    nc = _build_nc(N_CH, H, W, R, CHUNK, n_img=n_img)
    in_maps = [{"all": packed[k]} for k in range(N_CORES)]
    res = run_bass_kernel_spmd(nc, in_maps, list(range(N_CORES)))
    delta = np.stack(
        [res.results[k]["out"].astype(np.float32) for k in range(N_CORES)], axis=0
    ).reshape(n, N_CH, H, W)
    return postprocess(delta, u, a, b, c, tau, grad_x)


# revision 17
# speedup vs baseline: 3.5231x; 1.0058x over previous
"""Trainium2 Bass kernel for nn_DiffusionBlock (anisotropic diffusion step).

Math (per batch-channel image; s = tau*hx^2, hx = grad kernel tap):
  X[i,j] = u[i,j+1]-u[i,j] (0 at j=W-1),  Y[i,j] = u[i+1,j]-u[i,j] (0 at i=H-1)
  XP/YP  = edge-pad(X/Y) on the (H+2, W+2) grid
  F = a*XP + b*YP,  G = b*XP + c*YP              (padded grid)
  out[i,j] = u[i,j] + s*(F[i+1,j+1]-F[i+1,j] + G[i+1,j+1]-G[i,j+1])

Pure batch data-parallel across 8 cores (1 batch each). The per-call cost
through the axon tunnel is dominated by per-operand dispatch overhead and
shipped input bytes, so ALL inputs are packed into ONE fp8 E3M4 tensor of
1024-byte rows (tolerance 2e-2, measured ~5e-3):
  rows [0, 2048):      u (ch-major image rows)
  rows [2048, +2052*3): a, b, c planes, padded-grid cols 1..W only
                        (b in [0,0.1] sits in E3M4's fine subnormal range)
  rows [8204, 8332):    3 PE weight matrices, bf16 bytes bitcast-loaded
The device computes the diffusion DELTA (s * divergence) in bf16; the host
adds the exact f32 u and recomputes output column 0 exactly (that column
needs a/b at padded col 0, which the packed planes drop).

Per-core layout: row-tiles of R=126 output rows. SBUF partition q holds:
  U[q]  = u row r0-1+q (top edge-clamped)     [rt+1, W]  fp8
  U2[q] = u row r0+q   (bottom edge-clamped)  [rt+1, W]  fp8
  A/B/C[q] = plane row r0+q (padded cols 1..W)[rt+1, W]  fp8
All gradients/products on DVE (partition-aligned):
  XT = free-dim diff of U;  YT = U2 - U (partition-offset loads)
  F[q,s] = A*XT + B*YT at padded row r0+q, cols 1..W (col 0 zeroed)
  G2[q,j] = G[r0+q, j+1] = B*XT + C*YT
PE assembles the divergence in PSUM with 3 constant bf16 weight matrices
(partition shift / sign / scale s folded in):
  DELTA[p] = s*(F[p+1]@j+1 - F[p+1]@j + G2[p+1] - G2[p])
ACT copies PSUM -> bf16, DMA stores. Top/bottom clamps are folded into the
DMA row loads (replicated rows); the first tile fixes YT[0] = YT[1] with a
1-partition SBUF copy.
"""

import numpy as np
import ml_dtypes

# Problem geometry (hardcoded per harness contract).
# N_CORES=1 measured fastest: per-call dispatch overhead through the axon
# tunnel scales with core count (~0.3 ms/core), the tunnel byte-pipe is
# shared, and device execution (~0.5 ms for all 8 images) hides entirely
# behind the ~4 ms dispatch+transfer pipeline. 8/4/2/1-core sweep gave
# 6.2/5.5/4.5/4.3 ms marginal per call.
N_CORES = 1
N_CH = 2
H = 1024
W = 1024
R = 126       # output rows per tile
CHUNK = 512   # matmul free-dim chunk (= one PSUM bank of fp32)

BF16 = ml_dtypes.bfloat16
F8E3 = ml_dtypes.float8_e3m4

_W_NAMES = ("wsp", "wsn", "wg")
# packed-tensor row bases
ROW_A = N_CH * H
ROW_B = ROW_A + N_CH * (H + 2)
ROW_C = ROW_B + N_CH * (H + 2)
ROW_W = ROW_C + N_CH * (H + 2)
N_ROWS = ROW_W + 128


def _host_weights(s: float):
    """Constant PE weight matrices, packed [128, 3*128] bf16.

    matmul(out, lhsT, rhs): out[p, n] = sum_k lhsT[k, p] * rhs[k, n]
    """
    k = np.arange(128)[:, None]
    p = np.arange(128)[None, :]
    sf = np.float32(s)
    wsp = sf * (k == p + 1)                  # out[p] += s * x[p+1]
    wsn = -sf * (k == p + 1)                 # out[p] -= s * x[p+1]
    wg = sf * (k == p + 1) - sf * (k == p)   # out[p] += s * (x[p+1]-x[p])
    mats = {"wsp": wsp, "wsn": wsn, "wg": wg}
    return np.ascontiguousarray(
        np.concatenate([mats[n].astype(np.float32) for n in _W_NAMES], axis=1)
    ).astype(BF16)


def _build_nc(n_ch: int, h: int, w: int, r: int, chunk: int, n_img: int = 1,
              reps: int = 1, mode: str = "full", out_fp8: bool = False):
    import concourse.bacc as bacc
    import concourse.mybir as mybir
    import concourse.tile as tile

    f32 = mybir.dt.float32
    bf16 = mybir.dt.bfloat16
    f8e3 = mybir.dt.float8e3
    out_dt = f8e3 if out_fp8 else bf16

    nwt = len(_W_NAMES) * 128
    nc = bacc.Bacc(enable_partition_id=False)
    all_d = nc.dram_tensor(
        "all", [n_img * ROW_W + 128, w], f8e3, kind="ExternalInput"
    )
    out_d = nc.dram_tensor("out", [n_img * n_ch, h, w], out_dt, kind="ExternalOutput")

    tiles = [(r0, min(r, h - r0)) for r0 in range(0, h, r)]

    with tile.TileContext(nc) as tc:
        with (
            tc.tile_pool(name="wpool", bufs=1) as wpool,
            tc.tile_pool(name="io", bufs=3) as io,
            tc.tile_pool(name="tmp", bufs=2) as tmp,
            tc.tile_pool(name="psum", bufs=2, space="PSUM") as psum,
        ):
            # one DMA for the weights (fp8 rows bitcast back to bf16), then a
            # tiny high-priority matmul so PE observes the weights DMA once up
            # front (matmul sync-wait slots are scarce)
            w_all = wpool.tile([128, nwt], bf16, tag="w_all")
            wbase = n_img * ROW_W
            nc.sync.dma_start(
                w_all[:], all_d[wbase : wbase + 128, 0 : 2 * nwt].bitcast(bf16)
            )
            wt = {
                n: w_all[:, i * 128 : (i + 1) * 128]
                for i, n in enumerate(_W_NAMES)
            }
            warm = psum.tile([1, 4], f32, tag="DELTA")
            with tc.high_priority():
                nc.tensor.matmul(warm[0:1, 0:1], w_all[0:1, 0:1], w_all[0:1, 0:1])

            for _rep in range(reps):
              for img in range(n_img):
               ib = img * ROW_W
               for ch in range(n_ch):
                for r0, rt in tiles:
                    first = r0 == 0
                    last = r0 + rt == h
                    ka = rt + 1      # working partitions
                    ub = ib + ch * h
                    # ---- loads ----
                    # U[q] = u row r0-1+q (top-clamped)
                    U = io.tile([128, w], f8e3, tag="U")
                    lo = r0 - 1
                    clo = max(lo, 0)
                    nc.sync.dma_start(
                        U[clo - lo : ka, :], all_d[ub + clo : ub + lo + ka, :]
                    )
                    if first:
                        nc.sync.dma_start(U[0:1, :], all_d[ub : ub + 1, :])
                    # U2[q] = u row r0+q (bottom-clamped)
                    U2 = io.tile([128, w], f8e3, tag="U2")
                    hi = min(r0 + ka, h)
                    nc.sync.dma_start(
                        U2[0 : hi - r0, :], all_d[ub + r0 : ub + hi, :]
                    )
                    if last:
                        nc.sync.dma_start(
                            U2[ka - 1 : ka, :],
                            all_d[ub + h - 1 : ub + h, :],
                        )
                    # plane rows r0..r0+rt (padded-grid cols 1..W)
                    hp = h + 2
                    ab = ib + ROW_A + ch * hp + r0
                    bb = ib + ROW_B + ch * hp + r0
                    cb = ib + ROW_C + ch * hp + r0
                    A = io.tile([128, w], f8e3, tag="A")
                    Bt = io.tile([128, w], f8e3, tag="B")
                    C = io.tile([128, w], f8e3, tag="C")
                    nc.sync.dma_start(A[0:ka, :], all_d[ab : ab + ka, :])
                    nc.sync.dma_start(Bt[0:ka, :], all_d[bb : bb + ka, :])
                    nc.sync.dma_start(C[0:ka, :], all_d[cb : cb + ka, :])

                    do_dve = mode in ("full", "nomm")
                    do_pe = mode in ("full", "nodve")
                    # ---- gradients (DVE) ----
                    # XT[q] = X row r0-1+q: free-dim forward diff, col W-1 = 0
                    XT = tmp.tile([128, w], bf16, tag="XT")
                    YT = tmp.tile([128, w], bf16, tag="YT")
                    if do_dve:
                        nc.vector.tensor_sub(
                            XT[0:ka, 0 : w - 1], U[0:ka, 1:w], U[0:ka, 0 : w - 1]
                        )
                        nc.vector.memset(XT[0:ka, w - 1 : w], 0.0)
                        # YT[q] = Y row r0-1+q = U2 - U (partition-offset loads)
                        nc.vector.tensor_sub(YT[0:ka, :], U2[0:ka, :], U[0:ka, :])
                        if first:
                            # YT[0] = Y[-1] -> clamp = Y[0] (= YT[1])
                            nc.sync.dma_start(YT[0:1, :], YT[1:2, :])

                    # ---- products (DVE) ----
                    # F[q,s] = a[r0+q,s]*XT[q,s-1] + b[r0+q,s]*YT[q,s-1],
                    #   s in 1..W (planes hold cols 1..W at local col s-1);
                    #   F col 0 zeroed (host recomputes out col 0 exactly)
                    F = tmp.tile([128, w + 1], bf16, tag="F")
                    T = tmp.tile([128, w], bf16, tag="T")
                    G2 = tmp.tile([128, w], bf16, tag="G2")
                    T2 = tmp.tile([128, w], bf16, tag="T2")
                    if do_dve:
                        nc.vector.tensor_mul(F[0:ka, 1 : w + 1], A[0:ka, :], XT[0:ka, :])
                        nc.vector.memset(F[0:ka, 0:1], 0.0)
                        nc.vector.tensor_mul(T[0:ka, :], Bt[0:ka, :], YT[0:ka, :])
                        nc.vector.tensor_add(
                            F[0:ka, 1 : w + 1], F[0:ka, 1 : w + 1], T[0:ka, :]
                        )
                        # G2[q,j] = G[r0+q, j+1]
                        nc.vector.tensor_mul(G2[0:ka, :], Bt[0:ka, :], XT[0:ka, :])
                        nc.vector.tensor_mul(T2[0:ka, :], C[0:ka, :], YT[0:ka, :])
                        nc.vector.tensor_add(G2[0:ka, :], G2[0:ka, :], T2[0:ka, :])

                    # ---- PSUM assembly (PE) ----
                    # DELTA[p] = s*(F[p+1]@j+1 - F[p+1]@j + G2[p+1] - G2[p])
                    DELTA = psum.tile([128, w], f32, tag="DELTA")
                    for n0 in (range(0, w, chunk) if do_pe else ()):
                        cw = min(chunk, w - n0)
                        o = DELTA[0:rt, n0 : n0 + cw]
                        mm = [
                            (wt["wsp"][0:ka, 0:rt], F[0:ka, n0 + 1 : n0 + 1 + cw]),
                            (wt["wsn"][0:ka, 0:rt], F[0:ka, n0 : n0 + cw]),
                            (wt["wg"][0:ka, 0:rt], G2[0:ka, n0 : n0 + cw]),
                        ]
                        for i, (lhsT, rhs) in enumerate(mm):
                            nc.tensor.matmul(
                                o,
                                lhsT,
                                rhs,
                                start=(i == 0),
                                stop=(i == len(mm) - 1),
                            )

                    # ---- PSUM -> SBUF (ACT), store ----
                    OS = tmp.tile([128, w], out_dt, tag="OS")
                    if do_pe and not do_dve:
                        for _t in (F, G2):
                            nc.vector.memset(_t[0:128, :], 0.0)
                    if do_pe:
                        nc.scalar.copy(OS[0:rt, :], DELTA[0:rt, :])
                    else:
                        nc.vector.memset(OS[0:128, :], 0.0)
                    nc.sync.dma_start(
                        out_d[img * n_ch + ch, r0 : r0 + rt, :], OS[0:rt, :]
                    )

    nc.compile()
    return nc


def _scale(tau, grad_x):
    hx = float(np.asarray(grad_x)[0, 0, 1, 2])
    return float(np.asarray(tau)) * hx * hx


def prepare_inputs(u, a, b, c, tau, grad_x, grad_y, n_cores: int = None):
    """Host casts + packing into one fp8 tensor [n_cores, n_img*ROW_W+128, W]."""
    if n_cores is None:
        n_cores = N_CORES
    s = _scale(tau, grad_x)
    wts = _host_weights(s)
    n = np.asarray(u).shape[0]
    n_img = n // n_cores
    u8 = np.ascontiguousarray(np.asarray(u, dtype=np.float32)).astype(F8E3)
    a8 = np.ascontiguousarray(np.asarray(a, np.float32)[:, :, :, 1 : W + 1]).astype(F8E3)
    b8 = np.ascontiguousarray(np.asarray(b, np.float32)[:, :, :, 1 : W + 1]).astype(F8E3)
    c8 = np.ascontiguousarray(np.asarray(c, np.float32)[:, :, :, 1 : W + 1]).astype(F8E3)
    wrow = np.zeros((128, W), F8E3)
    wrow[:, : 2 * wts.shape[1]] = np.ascontiguousarray(wts).view(F8E3)
    # per-image data block [ROW_W, W]
    data = np.concatenate(
        [
            u8.reshape(n, N_CH * H, W),
            a8.reshape(n, N_CH * (H + 2), W),
            b8.reshape(n, N_CH * (H + 2), W),
            c8.reshape(n, N_CH * (H + 2), W),
        ],
        axis=1,
    )  # [n, ROW_W, W]
    # core k gets images k*n_img .. (k+1)*n_img-1, then the weight block
    packed = np.concatenate(
        [
            data.reshape(n_cores, n_img * ROW_W, W),
            np.broadcast_to(wrow, (n_cores, 128, W)),
        ],
        axis=1,
    )
    return np.ascontiguousarray(packed)


def postprocess(delta_f32, u, a, b, c, tau, grad_x):
    """out = u + delta, with output column 0 recomputed exactly on host."""
    s = _scale(tau, grad_x)
    u = np.asarray(u, np.float32)
    a = np.asarray(a, np.float32)
    b = np.asarray(b, np.float32)
    c = np.asarray(c, np.float32)
    out = u + delta_f32
    X0 = u[..., 1] - u[..., 0]                        # [B, C, H]
    Y0 = np.zeros_like(X0)
    Y0[..., : H - 1] = u[..., 1:, 0] - u[..., : H - 1, 0]
    rr = np.clip(np.arange(H + 2) - 1, 0, H - 1)
    Xp0, Yp0 = X0[..., rr], Y0[..., rr]               # [B, C, H+2]
    F0 = a[..., 0] * Xp0 + b[..., 0] * Yp0
    F1 = a[..., 1] * Xp0 + b[..., 1] * Yp0
    G1 = b[..., 1] * Xp0 + c[..., 1] * Yp0
    out[..., 0] = u[..., 0] + s * (
        F1[..., 1 : H + 1] - F0[..., 1 : H + 1] + G1[..., 1 : H + 1] - G1[..., 0:H]
    )
    return out


def kernel(u, a, b, c, tau, grad_x, grad_y):
    from concourse.bass_utils import run_bass_kernel_spmd

    n = np.asarray(u).shape[0]
    n_img = n // N_CORES
    packed = prepare_inputs(u, a, b, c, tau, grad_x, grad_y, N_CORES)
    nc = _build_nc(N_CH, H, W, R, CHUNK, n_img=n_img)
    in_maps = [{"all": packed[k]} for k in range(N_CORES)]
    res = run_bass_kernel_spmd(nc, in_maps, list(range(N_CORES)))
    delta = np.stack(
        [res.results[k]["out"].astype(np.float32) for k in range(N_CORES)], axis=0
    ).reshape(n, N_CH, H, W)
    return postprocess(delta, u, a, b, c, tau, grad_x)


# revision 20
# speedup vs baseline: 9.1369x; 2.5934x over previous
"""Trainium2 Bass kernel for nn_DiffusionBlock (anisotropic diffusion step).

Math (per batch-channel image; s = tau*hx^2, hx = grad kernel tap):
  X[i,j] = u[i,j+1]-u[i,j] (0 at j=W-1),  Y[i,j] = u[i+1,j]-u[i,j] (0 at i=H-1)
  XP/YP  = edge-pad(X/Y) on the (H+2, W+2) grid
  F = a*XP + b*YP,  G = b*XP + c*YP              (padded grid)
  out[i,j] = u[i,j] + s*(F[i+1,j+1]-F[i+1,j] + G[i+1,j+1]-G[i,j+1])

The per-call cost through the axon tunnel is dominated by (1) per-operand
dispatch overhead, (2) shipped input bytes (~20 GB/s), (3) the device's DMA
fixed costs; compute engines are idle by comparison. So:
  - ONE core runs all 8 batch images (dispatch overhead scales with cores,
    the tunnel pipe is shared, device compute hides under it).
  - ONE input tensor of 512-byte uint8 rows (dma_start only needs equal
    element counts, so row-groups bitcast to wider tiles):
      per image: u as fp8 E3M4 (2 rows per image row),
                 a|b|c as packed 4-bit codes (3 rows per grid row,
                 value = code/15*max, planes' padded cols 1..W only)
      tail: my/myf/myl/myfl PE matrices as E3M4 (exact; +-1 entries),
            wsp/wsn/wg as bf16 bytes (exact; +-s entries)
  - 3 DMAs per row-tile (u, abc, store) instead of 6.
Tolerance is 2e-2; this config measures ~9e-3 (fp8 u gradients + 4-bit
coefficient quantization).

Per-core layout: row-tiles of R=126 output rows (9 tiles x 2 ch x n_img).
SBUF partition q holds:
  U[q] = u row r0-1+q (top edge-clamped, E3M4)  [rt+2, W]
  ABC[q] = packed a|b|c row r0+q (uint8)        [rt+1, 1536]
DVE decodes nibbles (A/B/C bf16 = (raw & 0xF|0xF0) * k), computes
  XT = free-dim diff of U (col W-1 = 0)
  F[q,s] = A*XT + B*YT at padded row r0+q, cols 1..W (col 0 zeroed)
  G2[q,j] = G[r0+q, j+1] = B*XT + C*YT
PE computes YT[q] = Y row r0-1+q = U[q+1]-U[q] via bidiagonal E3M4
matmuls (first/last-tile clamps folded into myf/myl variants), then
assembles the divergence in PSUM with bf16 weights (shift/sign/scale
folded):  DELTA[p] = s*(F[p+1]@j+1 - F[p+1]@j + G2[p+1] - G2[p])
ACT copies PSUM -> bf16, DMA stores the delta. The host adds the exact
f32 u and recomputes output column 0 exactly (needs a/b at padded col 0,
which the packed planes drop).
"""

import numpy as np
import ml_dtypes

# Problem geometry (hardcoded per harness contract).
N_CORES = 1
N_CH = 2
H = 1024
W = 1024
R = 126       # output rows per tile
CHUNK = 512   # matmul free-dim chunk (= one PSUM bank of fp32)

BF16 = ml_dtypes.bfloat16
F8E3 = ml_dtypes.float8_e3m4

# single input tensor: 512-byte uint8 rows
ROW_BYTES = 512
U_ROWS = N_CH * H * 2              # 2 rows per 1024-byte u image row
ABC_ROWS = N_CH * (H + 2) * 3      # 3 rows per 1536-byte packed a|b|c row
IMG_ROWS = U_ROWS + ABC_ROWS       # 10252
W_MY_ROWS = 128                    # my|myf|myl|myfl, E3M4 [128, 512]
W_SB_ROWS = 256                    # wsp|wsn|wg bf16 [128, 512] = 2 rows each
_MY_NAMES = ("my", "myf", "myl", "myfl")
_SB_NAMES = ("wsp", "wsn", "wg")


def _host_weights(s: float, rt_last: int):
    """PE weight matrices: (wmy [128,512] E3M4-exact, wsb [128,512] bf16).

    matmul(out, lhsT, rhs): out[p, n] = sum_k lhsT[k, p] * rhs[k, n]
    """
    k = np.arange(128)[:, None]
    p = np.arange(128)[None, :]
    sf = np.float32(s)
    my = (k == p + 1).astype(np.float32) - (k == p)  # YT[q] = U[q+1]-U[q]
    myf = my.copy()                                  # first tile: YT[0] = U[2]-U[1]
    myf[:, 0] = 0.0
    myf[2, 0] = 1.0
    myf[1, 0] = -1.0
    myl = my.copy()                                  # last tile: YT[rt] = 0
    myl[:, rt_last] = 0.0
    myfl = myf.copy()
    myfl[:, rt_last] = 0.0
    wmy = np.concatenate([my, myf, myl, myfl], axis=1).astype(F8E3)  # exact
    wsp = sf * (k == p + 1)                  # out[p] += s * x[p+1]
    wsn = -sf * (k == p + 1)                 # out[p] -= s * x[p+1]
    wg = sf * (k == p + 1) - sf * (k == p)   # out[p] += s * (x[p+1]-x[p])
    wsb = np.zeros((128, 512), np.float32)
    wsb[:, 0:384] = np.concatenate([wsp, wsn, wg], axis=1)
    return np.ascontiguousarray(wmy), np.ascontiguousarray(wsb.astype(BF16))


def _build_nc(n_ch: int, h: int, w: int, r: int, chunk: int, scales,
              n_img: int = 1, reps: int = 1, mode: str = "full"):
    import concourse.bacc as bacc
    import concourse.mybir as mybir
    import concourse.tile as tile

    f32 = mybir.dt.float32
    bf16 = mybir.dt.bfloat16
    f8e3 = mybir.dt.float8e3
    u8 = mybir.dt.uint8
    and_op = mybir.AluOpType.bitwise_and
    mult_op = mybir.AluOpType.mult

    nc = bacc.Bacc(enable_partition_id=False)
    wb = n_img * IMG_ROWS
    all_d = nc.dram_tensor(
        "all", [wb + W_MY_ROWS + W_SB_ROWS, ROW_BYTES], u8, kind="ExternalInput"
    )
    out_d = nc.dram_tensor("out", [n_img * n_ch, h, w], bf16, kind="ExternalOutput")

    tiles = [(r0, min(r, h - r0)) for r0 in range(0, h, r)]

    with tile.TileContext(nc) as tc:
        with (
            tc.tile_pool(name="wpool", bufs=1) as wpool,
            tc.tile_pool(name="io", bufs=3) as io,
            tc.tile_pool(name="tmp", bufs=2) as tmp,
            tc.tile_pool(name="psum", bufs=2, space="PSUM") as psum,
        ):
            # weight loads (one-time), then a tiny high-priority matmul so PE
            # observes the weights DMA once up front (matmul sync-wait slots
            # are scarce)
            wmy = wpool.tile([128, 512], f8e3, tag="wmy")
            nc.sync.dma_start(wmy[:], all_d[wb : wb + 128, :].bitcast(f8e3))
            wsb = wpool.tile([128, 512], bf16, tag="wsb")
            nc.sync.dma_start(
                wsb[:], all_d[wb + 128 : wb + 384, :].bitcast(bf16)
            )
            myv = {
                n: wmy[:, i * 128 : (i + 1) * 128]
                for i, n in enumerate(_MY_NAMES)
            }
            wt = {
                n: wsb[:, i * 128 : (i + 1) * 128]
                for i, n in enumerate(_SB_NAMES)
            }
            warm = psum.tile([1, 4], f32, tag="YT")
            with tc.high_priority():
                nc.tensor.matmul(warm[0:1, 0:1], wmy[0:1, 0:1], wmy[0:1, 0:1])

            (sa, sb, sc) = scales
            for _rep in range(reps):
              for img in range(n_img):
               for ch in range(n_ch):
                for r0, rt in tiles:
                    first = r0 == 0
                    last = r0 + rt == h
                    ka = rt + 1      # working partitions
                    ku = rt + 1 if last else rt + 2  # loaded U partitions
                    # ---- loads (u rows are 2 tensor-rows each) ----
                    ub = img * IMG_ROWS + ch * h * 2
                    U = io.tile([128, w], f8e3, tag="U")
                    lo = r0 - 1
                    clo = max(lo, 0)
                    nc.sync.dma_start(
                        U[clo - lo : ku, :],
                        all_d[ub + 2 * clo : ub + 2 * (lo + ku), :].bitcast(f8e3),
                    )
                    if first:
                        nc.sync.dma_start(
                            U[0:1, :], all_d[ub : ub + 2, :].bitcast(f8e3)
                        )
                    # packed a|b|c rows r0..r0+rt (3 tensor-rows per grid row)
                    ab = img * IMG_ROWS + U_ROWS + (ch * (h + 2) + r0) * 3
                    ABC = io.tile([128, 3 * ROW_BYTES], u8, tag="ABC")
                    nc.sync.dma_start(
                        ABC[0:ka, :], all_d[ab : ab + 3 * ka, :]
                    )

                    do_dve = mode in ("full", "nomm")
                    do_pe = mode in ("full", "nodve")
                    # ---- YT (PE): partition-dim forward diff -> PSUM ----
                    # YT[q] = Y row r0-1+q = U[q+1] - U[q] (edge variants folded)
                    YT = psum.tile([128, w], f32, tag="YT")
                    my = myv[{(0, 0): "my", (1, 0): "myf",
                              (0, 1): "myl", (1, 1): "myfl"}[(first, last)]]
                    for n0 in (range(0, w, chunk) if do_pe else ()):
                        nc.tensor.matmul(
                            YT[0:ka, n0 : n0 + chunk],
                            my[0:ku, 0:ka],
                            U[0:ku, n0 : n0 + chunk],
                        )

                    # ---- decode 4-bit planes (DVE): val = (raw&mask)*k ----
                    A = tmp.tile([128, w], bf16, tag="A")
                    Bt = tmp.tile([128, w], bf16, tag="B")
                    C = tmp.tile([128, w], bf16, tag="C")
                    XT = tmp.tile([128, w], bf16, tag="XT")
                    if do_dve:
                        for dst, base, s_pl in ((A, 0, sa), (Bt, 512, sb), (C, 1024, sc)):
                            raw = ABC[:, base : base + 512]
                            for hi, mask in ((0, 15), (1, 240)):
                                nib = tmp.tile([128, 512], u8, tag="NIB")
                                nc.vector.tensor_scalar(
                                    nib[0:ka, :], raw[0:ka, :], mask, None, and_op
                                )
                                nc.vector.tensor_scalar(
                                    dst[0:ka, hi * 512 : (hi + 1) * 512],
                                    nib[0:ka, :], float(s_pl / (15.0 * (16 if hi else 1))),
                                    None, mult_op,
                                )
                        # XT[q] = X row r0-1+q: free-dim diff, col W-1 = 0
                        nc.vector.tensor_sub(
                            XT[0:ka, 0 : w - 1], U[0:ka, 1:w], U[0:ka, 0 : w - 1]
                        )
                        nc.vector.memset(XT[0:ka, w - 1 : w], 0.0)

                    # ---- products (DVE) ----
                    # F[q,s] = a[r0+q,s]*XT[q,s-1] + b[r0+q,s]*YT[q,s-1],
                    #   s in 1..W (planes hold cols 1..W); col 0 zeroed
                    #   (host recomputes out col 0 exactly)
                    F = tmp.tile([128, w + 1], bf16, tag="F")
                    T = tmp.tile([128, w], bf16, tag="T")
                    G2 = tmp.tile([128, w], bf16, tag="G2")
                    T2 = tmp.tile([128, w], bf16, tag="T2")
                    if do_dve and do_pe:
                        nc.vector.tensor_mul(F[0:ka, 1 : w + 1], A[0:ka, :], XT[0:ka, :])
                        nc.vector.memset(F[0:ka, 0:1], 0.0)
                        nc.vector.tensor_mul(T[0:ka, :], Bt[0:ka, :], YT[0:ka, :])
                        nc.vector.tensor_add(
                            F[0:ka, 1 : w + 1], F[0:ka, 1 : w + 1], T[0:ka, :]
                        )
                        # G2[q,j] = G[r0+q, j+1]
                        nc.vector.tensor_mul(G2[0:ka, :], Bt[0:ka, :], XT[0:ka, :])
                        nc.vector.tensor_mul(T2[0:ka, :], C[0:ka, :], YT[0:ka, :])
                        nc.vector.tensor_add(G2[0:ka, :], G2[0:ka, :], T2[0:ka, :])
                    elif do_dve:
                        nc.vector.memset(F[0:128, :], 0.0)
                        nc.vector.memset(G2[0:128, :], 0.0)

                    # ---- PSUM assembly (PE) ----
                    # DELTA[p] = s*(F[p+1]@j+1 - F[p+1]@j + G2[p+1] - G2[p])
                    DELTA = psum.tile([128, w], f32, tag="DELTA")
                    for n0 in (range(0, w, chunk) if do_pe and do_dve else ()):
                        cw = min(chunk, w - n0)
                        o = DELTA[0:rt, n0 : n0 + cw]
                        mm = [
                            (wt["wsp"][0:ka, 0:rt], F[0:ka, n0 + 1 : n0 + 1 + cw]),
                            (wt["wsn"][0:ka, 0:rt], F[0:ka, n0 : n0 + cw]),
                            (wt["wg"][0:ka, 0:rt], G2[0:ka, n0 : n0 + cw]),
                        ]
                        for i, (lhsT, rhs) in enumerate(mm):
                            nc.tensor.matmul(
                                o,
                                lhsT,
                                rhs,
                                start=(i == 0),
                                stop=(i == len(mm) - 1),
                            )

                    # ---- PSUM -> SBUF bf16 (ACT), store ----
                    OS = tmp.tile([128, w], bf16, tag="OS")
                    if do_pe and do_dve:
                        nc.scalar.copy(OS[0:rt, :], DELTA[0:rt, :])
                    else:
                        nc.vector.memset(OS[0:128, :], 0.0)
                    nc.sync.dma_start(
                        out_d[img * n_ch + ch, r0 : r0 + rt, :], OS[0:rt, :]
                    )

    nc.compile()
    return nc


def _scale(tau, grad_x):
    hx = float(np.asarray(grad_x)[0, 0, 1, 2])
    return float(np.asarray(tau)) * hx * hx


def _pack_plane(x):
    """f32 plane [..., 1024] (padded cols 1..W) -> (codes packed u8 [..., 512],
    scale). value = code/15*scale."""
    smax = float(x.max())
    if not np.isfinite(smax) or smax <= 0:
        smax = 1.0
    codes = np.clip(np.rint(x * (15.0 / smax)), 0, 15).astype(np.uint8)
    return codes[..., 0:512] | (codes[..., 512:1024] << 4), smax


def prepare_inputs(u, a, b, c, tau, grad_x, grad_y, n_cores: int = None):
    """Host casts + packing into one uint8 tensor [n_cores, rows, 512].

    Returns (packed, scales)."""
    if n_cores is None:
        n_cores = N_CORES
    s = _scale(tau, grad_x)
    rt_last = H % R if H % R else R
    wmy, wsb = _host_weights(s, rt_last)
    n = np.asarray(u).shape[0]
    n_img = n // n_cores
    u8 = np.ascontiguousarray(np.asarray(u, dtype=np.float32)).astype(F8E3)
    pa, sa = _pack_plane(np.asarray(a, np.float32)[:, :, :, 1 : W + 1])
    pb, sb = _pack_plane(np.asarray(b, np.float32)[:, :, :, 1 : W + 1])
    pc, sc = _pack_plane(np.asarray(c, np.float32)[:, :, :, 1 : W + 1])
    abc = np.concatenate([pa, pb, pc], axis=3)        # [n, ch, h+2, 1536]
    img_blocks = np.concatenate(
        [
            u8.view(np.uint8).reshape(n, U_ROWS, ROW_BYTES),
            abc.reshape(n, ABC_ROWS, ROW_BYTES),
        ],
        axis=1,
    )  # [n, IMG_ROWS, 512]
    wrows = np.concatenate(
        [
            wmy.view(np.uint8).reshape(W_MY_ROWS, ROW_BYTES),
            wsb.view(np.uint8).reshape(W_SB_ROWS, ROW_BYTES),
        ],
        axis=0,
    )  # [384, 512]
    packed = np.concatenate(
        [
            img_blocks.reshape(n_cores, n_img * IMG_ROWS, ROW_BYTES),
            np.broadcast_to(wrows, (n_cores, *wrows.shape)),
        ],
        axis=1,
    )
    return np.ascontiguousarray(packed), (sa, sb, sc)


def postprocess(delta_f32, u, a, b, c, tau, grad_x):
    """out = u + delta, with output column 0 recomputed exactly on host."""
    s = _scale(tau, grad_x)
    u = np.asarray(u, np.float32)
    a = np.asarray(a, np.float32)
    b = np.asarray(b, np.float32)
    c = np.asarray(c, np.float32)
    out = u + delta_f32
    X0 = u[..., 1] - u[..., 0]                        # [B, C, H]
    Y0 = np.zeros_like(X0)
    Y0[..., : H - 1] = u[..., 1:, 0] - u[..., : H - 1, 0]
    rr = np.clip(np.arange(H + 2) - 1, 0, H - 1)
    Xp0, Yp0 = X0[..., rr], Y0[..., rr]               # [B, C, H+2]
    F0 = a[..., 0] * Xp0 + b[..., 0] * Yp0
    F1 = a[..., 1] * Xp0 + b[..., 1] * Yp0
    G1 = b[..., 1] * Xp0 + c[..., 1] * Yp0
    out[..., 0] = u[..., 0] + s * (
        F1[..., 1 : H + 1] - F0[..., 1 : H + 1] + G1[..., 1 : H + 1] - G1[..., 0:H]
    )
    return out


def kernel(u, a, b, c, tau, grad_x, grad_y):
    from concourse.bass_utils import run_bass_kernel_spmd

    n = np.asarray(u).shape[0]
    n_img = n // N_CORES
    packed, scales = prepare_inputs(u, a, b, c, tau, grad_x, grad_y, N_CORES)
    nc = _build_nc(N_CH, H, W, R, CHUNK, scales, n_img=n_img)
    in_maps = [{"all": packed[k]} for k in range(N_CORES)]
    res = run_bass_kernel_spmd(nc, in_maps, list(range(N_CORES)))
    delta = np.stack(
        [res.results[k]["out"].astype(np.float32) for k in range(N_CORES)], axis=0
    ).reshape(n, N_CH, H, W)
    return postprocess(delta, u, a, b, c, tau, grad_x)
